# revision 12
# baseline (speedup 1.0000x reference)
"""Trainium2 Bass kernel for nn_MultiHeadAttention_54614804136658.

Forward pass of the reference collapses to: out = v + sum_h P_h[argmax_j(qh_h @ kh_h^T)]
where P_h = v @ (w_vs_h @ w_fc_h), because the straight-through estimator
(hard - stop_grad(attn) + attn) makes the forward attention an exact one-hot of
the score argmax (softmax/topk/scale are monotonic and keep the max).

Sharding: 8 cores = 2 batches x 4 head-groups (2 heads each). Per core:
  A: khT fp32 projection streamed per kt DMA chunk (PE pre-warmed with junk
     matmuls so projections run at full pstate); qhT projected JIT per
     512-column chunk as qt column-chunk DMAs land
  B: P_h = v @ W_h in bf16, interleaved into the steady loop's PE slack,
     PSUM evacuated by ACT -> pscr (DRAM, bf16)
  steady loop over 32 (h,t) score tiles (DVE-bound at ~4.4us/tile):
    PE: 2 half-tile fp32 matmuls [128,1024] into PSUM
    ACT: PSUM -> SBUF copy + batched iscr writeback (wrapped layout)
    DVE: max8 + max_index over [128,2048] SBUF + idx16 write
  D: per head, idxw readback (contiguous 256B rows) + dma_gather of P rows
     (bf16); h0 overlaps h1 compute; h1 in 12+4 chunks to cut the tail.
Host: fuses W = w_vs_h @ w_fc_h, transposes/slices inputs, sums partials + v.
"""
import numpy as np
from contextlib import ExitStack

B, L, E = 2, 2048, 512
H, DQK, DV = 8, 64, 256
QT = L // 128           # 16 query tiles
ETIL = E // 128         # 4 embed tiles

_CACHE = {}


def _build(phases="ABCD", num_devices=8):
    import concourse.bass as bass
    import concourse.tile as tile
    from concourse import bacc, mybir

    F32 = mybir.dt.float32
    BF16 = mybir.dt.bfloat16
    I16 = mybir.dt.int16
    U32 = mybir.dt.uint32

    nc = bacc.Bacc("TRN2", target_bir_lowering=False, debug=False,
                   num_devices=num_devices)
    dbg = num_devices == 1

    qt_d = nc.dram_tensor("qt", [E, L], F32, kind="ExternalInput").ap()
    kt_d = nc.dram_tensor("kt", [E, L], F32, kind="ExternalInput").ap()
    vt_d = nc.dram_tensor("vt", [DV, L], BF16, kind="ExternalInput").ap()
    wq_d = nc.dram_tensor("wq", [E, 128], F32, kind="ExternalInput").ap()
    wk_d = nc.dram_tensor("wk", [E, 128], F32, kind="ExternalInput").ap()
    W_d = nc.dram_tensor("W", [2, DV, DV], BF16, kind="ExternalInput").ap()
    out_d = nc.dram_tensor("out", [2, L, DV], BF16, kind="ExternalOutput").ap()
    pscr = nc.dram_tensor("pscr", [2, L, DV], BF16,
                          kind="ExternalOutput" if dbg else "Internal").ap()
    iscr = nc.dram_tensor("iscr", [2, L], I16,
                          kind="ExternalOutput" if dbg else "Internal").ap()

    with tile.TileContext(nc) as tc, ExitStack() as ctx:
        keep = ctx.enter_context(tc.tile_pool(name="keep", bufs=1))
        qhT = keep.tile([128, L], F32, tag="qhT")   # 2 heads stacked 64+64
        khT = keep.tile([128, L], F32, tag="khT")
        idx16 = keep.tile([128, 2, QT], I16, tag="idx16")
        P_s = keep.tile([128, QT, 2, DV], BF16, tag="P")

        # ---------- input DMAs: few large transfers on 4 parallel queues ----
        Q4 = [nc.sync, nc.scalar, nc.gpsimd, nc.sync]
        ldK = ctx.enter_context(tc.tile_pool(name="ldK", bufs=1))
        wk_s = ldK.tile([128, ETIL, 128], F32, tag="wk")
        nc.sync.dma_start(wk_s[:], wk_d.rearrange("(t p) m -> p t m", p=128))
        ldQ = ctx.enter_context(tc.tile_pool(name="ldQ", bufs=1))
        wq_s = ldQ.tile([128, ETIL, 128], F32, tag="wq")
        nc.scalar.dma_start(wq_s[:], wq_d.rearrange("(t p) m -> p t m", p=128))
        kt_s = ldK.tile([128, ETIL, L], F32, tag="kt")
        qt_s = ldQ.tile([128, ETIL, L], F32, tag="qt")
        for cc in range(4):   # kt column-chunks: khT nb-group ready per chunk
            Q4[cc].dma_start(
                kt_s[:, :, cc * 512:(cc + 1) * 512],
                kt_d[:, cc * 512:(cc + 1) * 512]
                .rearrange("(t p) c -> p t c", p=128))
        for cc in range(4):
            Q4[cc].dma_start(
                qt_s[:, :, cc * 512:(cc + 1) * 512],
                qt_d[:, cc * 512:(cc + 1) * 512]
                .rearrange("(t p) c -> p t c", p=128))

        ldB = ctx.enter_context(tc.tile_pool(name="ldB", bufs=1))
        vt_s = ldB.tile([128, 2, L], BF16, tag="vt")
        nc.sync.dma_start(vt_s[:], vt_d.rearrange("(t p) n -> p t n", p=128))
        W_s = ldB.tile([128, ETIL // 2, 2, DV], BF16, tag="W")
        for h in range(2):
            nc.scalar.dma_start(W_s[:, :, h, :],
                                W_d[h].rearrange("(t p) m -> p t m", p=128))

        # ---------- phase A: khT, et-streamed, with PE pre-warm ----------
        with tc.tile_pool(name="psA", bufs=1, space="PSUM") as psA:
            pss = [psA.tile([128, 512], F32, tag=f"psA{nb}", name=f"psA{nb}")
                   for nb in range(4)]
            for et in range(ETIL):
                for nb in range(4):
                    nc.tensor.matmul(
                        pss[nb][:], wk_s[:, et, :],
                        kt_s[:, et, nb * 512:(nb + 1) * 512],
                        start=(et == 0), stop=(et == ETIL - 1))
            for nb in range(4):
                nc.scalar.copy(khT[:, nb * 512:(nb + 1) * 512], pss[nb][:])

        # ---------- steady loop (+ JIT qhT, B interleaved, D overlapped) ----
        if "C" in phases:
          with tc.tile_pool(name="scps", bufs=2, space="PSUM") as scps, \
               tc.tile_pool(name="psQ", bufs=1, space="PSUM") as psQ, \
               tc.tile_pool(name="psB", bufs=2, space="PSUM") as psB, \
               tc.tile_pool(name="ysb", bufs=2) as ysb, \
               tc.tile_pool(name="scsb", bufs=4) as scsb, \
               tc.tile_pool(name="gth", bufs=1) as gth:

            psq_cur = [None]

            def project_q_mm(cc, et):
                # one accumulation step of qhT columns [cc*512, (cc+1)*512)
                if et == 0:
                    psq_cur[0] = psQ.tile([128, 512], F32, tag="q", name="ps_q")
                psq = psq_cur[0]
                nc.tensor.matmul(
                    psq[:], wq_s[:, et, :],
                    qt_s[:, et, cc * 512:(cc + 1) * 512],
                    start=(et == 0), stop=(et == ETIL - 1))
                if et == ETIL - 1:
                    nc.scalar.copy(qhT[:, cc * 512:(cc + 1) * 512], psq[:])

            def project_q(cc):
                for et in range(ETIL):
                    project_q_mm(cc, et)

            def b_unit(rt):
                psb = psB.tile([128, 2 * DV], F32, tag="pb", name="ps_pb")
                for et in range(2):
                    nc.tensor.matmul(
                        psb[:], vt_s[:, et, rt * 128:(rt + 1) * 128],
                        W_s[:, et, :, :], start=(et == 0), stop=(et == 1))
                nc.scalar.copy(P_s[:, rt, :, :], psb[:])
                if rt == QT - 1:
                    for h in range(2):
                        nc.sync.dma_start(
                            pscr[h].rearrange("(t p) e -> p t e", p=128),
                            P_s[:, :, h, :])

            def iscr_write(h, t0, t1):
                nc.scalar.dma_start(
                    iscr[h].rearrange("(t p) -> p t", p=128)[:, t0:t1],
                    idx16[:, h, t0:t1])

            def gather(h, t0, t1):
                n_idx = (t1 - t0) * 128
                idxw = gth.tile([128, n_idx // 16], I16, tag=f"idxw{h}_{t0}")
                src = iscr[h][t0 * 128:t1 * 128].rearrange("(c p) -> p c", p=16)
                for r in range(8):
                    eng = [nc.sync, nc.scalar, nc.gpsimd, nc.scalar][r % 4]
                    eng.dma_start(idxw[16 * r:16 * (r + 1), :], src)
                g = gth.tile([128, t1 - t0, DV], BF16, tag=f"g{h}_{t0}")
                nc.gpsimd.dma_gather(
                    out_ap=g[:], in_ap=pscr[h], idxs_ap=idxw[:],
                    num_idxs=n_idx, num_idxs_reg=n_idx, elem_size=DV,
                    single_packet=False)
                nc.sync.dma_start(
                    out_d[h].rearrange("(t p) e -> p t e", p=128)[:, t0:t1, :],
                    g[:])

            project_q(0)
            for h in range(2):
                for t in range(QT):
                    if h == 0 and t < 12:
                        project_q_mm(t // 4 + 1, t % 4)
                    y = ysb.tile([128, L], F32, tag="y", name="y")
                    for half in range(2):
                        ps = scps.tile([128, 1024], F32, tag="sc", name="ps_sc")
                        for kb in range(2):
                            col = half * 1024 + kb * 512
                            nc.tensor.matmul(
                                ps[:, kb * 512:(kb + 1) * 512],
                                qhT[h * 64:(h + 1) * 64, t * 128:(t + 1) * 128],
                                khT[h * 64:(h + 1) * 64, col:col + 512],
                                start=True, stop=True)
                        nc.scalar.copy(y[:, half * 1024:(half + 1) * 1024], ps[:])
                    if h == 0 and 3 <= t < 14:
                        for j in range(2 * (t - 3), min(2 * (t - 2), QT)):
                            b_unit(j)
                    m8 = scsb.tile([128, 8], F32, tag="m8")
                    nc.vector.max(m8[:], y[:])
                    i8 = scsb.tile([128, 8], U32, tag="i8")
                    nc.vector.max_index(i8[:], m8[:], y[:])
                    nc.vector.tensor_copy(idx16[:, h, t:t + 1], i8[:, 0:1])
                    if t % 4 == 3:
                        iscr_write(h, t - 3, t + 1)
                    if "D" in phases:
                        if h == 1 and t == QT - 5:
                            gather(1, 0, QT - 4)
                if "D" in phases and h == 0:
                    gather(0, 0, QT)
            if "D" in phases:
                gather(1, QT - 4, QT)

    nc.compile()
    return nc


def kernel(**inputs):
    from concourse.bass_utils import run_bass_kernel_spmd

    q = np.asarray(inputs["q"], np.float32)
    k = np.asarray(inputs["k"], np.float32)
    v = np.asarray(inputs["v"], np.float32)
    w_qs = np.asarray(inputs["w_qs"], np.float32)
    w_ks = np.asarray(inputs["w_ks"], np.float32)
    w_vs = np.asarray(inputs["w_vs"], np.float32)
    w_fc = np.asarray(inputs["w_fc"], np.float32)

    if "nc" not in _CACHE:
        _CACHE["nc"] = _build()
    nc = _CACHE["nc"]

    import ml_dtypes
    bf16 = ml_dtypes.bfloat16

    # fused per-head value->output projection
    W = np.empty((H, DV, DV), np.float32)
    for h in range(H):
        W[h] = (w_vs[:, h * DV:(h + 1) * DV].astype(np.float64)
                @ w_fc[h * DV:(h + 1) * DV, :].astype(np.float64)).astype(np.float32)

    in_maps = []
    for c in range(8):
        b, g = divmod(c, 4)
        in_maps.append({
            "qt": np.ascontiguousarray(q[b].T),
            "kt": np.ascontiguousarray(k[b].T),
            "vt": np.ascontiguousarray(v[b].T).astype(bf16),
            "wq": np.ascontiguousarray(w_qs[:, g * 128:(g + 1) * 128]),
            "wk": np.ascontiguousarray(w_ks[:, g * 128:(g + 1) * 128]),
            "W": np.ascontiguousarray(W[2 * g:2 * g + 2]).astype(bf16),
        })

    res = run_bass_kernel_spmd(nc, in_maps, core_ids=list(range(8)))
    _CACHE["last_result"] = res

    out = np.array(v)  # residual
    for c in range(8):
        b = c // 4
        co = res.results[c]["out"]
        out[b] += np.asarray(co[0], np.float32)
        out[b] += np.asarray(co[1], np.float32)
    return out
